# revision 22
# baseline (speedup 1.0000x reference)
"""Causal multi-head self-attention with RoPE on 8 TRN2 NeuronCores.

Sharding: batch(4) x head-group(2) -> 8 cores. Core c handles batch c//2 and
heads [8*(c%2), 8*(c%2)+8). Each core computes its partial output projection
(sum over its 8 heads' contribution); the host adds the two head-group
partials per batch. No device collectives needed.

On-chip layout: sequence lives on the free dimension everywhere.
  - Q^T/K^T [j, s] come straight out of the projection matmuls
    (lhsT = W^T slices, rhs = x^T), RoPE applied with a partition pair-swap
    (stream_shuffle) + precomputed cos/sin tables. The RoPE elementwise work
    is split DVE (shuffle + sin-mul) / GpSimd (cos-mul + add) so phase A is
    PE-bound rather than DVE-bound.
  - scores are computed transposed S^T = K^T.T-contraction -> [k, q] tiles
    into a merged [128, 2, SC] PSUM tile (both heads of the pair), exp on
    ScalarE in one instruction per k-tile (no max subtraction needed:
    |scores| <= ~15), causal masking by adding -1e30 on diagonal tiles
    (GpSimd) before exp.
  - P@V contraction runs over k on partitions; a ones-row appended to V
    makes the softmax denominator fall out of the same matmul (M=65).
  - output projection contracts the 512 head-dims -> partial y^T [1024, s].

Engine budget per core (f32r, TimelineSim cost model): PE ~235us is the
floor; exp on ScalarE ~140us; DVE and GpSimd each well under; DMA ~90us.
Startup orders DMAs (wv, x-chunk first) so the first matmul starts ~9us in.
"""

import os
import sys
import time

for _p in ("/opt/trn_rl_repo", "/root/.axon_site/_ro/trn_rl_repo"):
    if _p not in sys.path and os.path.isdir(_p):
        sys.path.insert(0, _p)

import numpy as np
import concourse.bass as bass
import concourse.bacc as bacc
import concourse.mybir as mybir
import concourse.tile as tile
from concourse.bass_utils import run_bass_kernel_spmd

F32 = mybir.dt.float32
F32R = mybir.dt.float32r
BF16 = mybir.dt.bfloat16

B, S, D = 4, 2048, 1024
H, DK = 16, 64
HPC = 8            # heads per core
JC = HPC * DK      # 512 head-dims per core
N_CORES = 8
SC = 512           # q-chunk width (moving free dim)
NSC = S // SC      # 4
KT = 128           # k-tile (scores partition dim)
NKT = S // KT      # 16
DT = D // 128      # 8 contraction tiles for projections

# matmul operand dtype: "f32" (exact) or "f32r" (tf32-like, ~4x faster PE)
MM_DTYPE = os.environ.get("KERNEL_MM_DTYPE", "f32r")
EBUFS = int(os.environ.get("KV_EBUFS", "5"))
SCBUFS = int(os.environ.get("KV_SCBUFS", "2"))
YBUFS = int(os.environ.get("KV_YBUFS", "2"))
APBUFS = int(os.environ.get("KV_APBUFS", "3"))
XBUFS = int(os.environ.get("KV_XBUFS", "2"))
RBUFS = int(os.environ.get("KV_RBUFS", "2"))
OBUFS = int(os.environ.get("KV_OBUFS", "2"))
ROPE_SPLIT = os.environ.get("KV_ROPE_SPLIT", "1") == "1"
VCOPY_ACT = os.environ.get("KV_VCOPY_ACT", "1") == "1"
MASK_GPS = os.environ.get("KV_MASK_GPS", "1") == "1"
PV_BF16 = os.environ.get("KV_PV_BF16", "1") == "1"


_PAIR_SWAP = []
for _i in range(16):
    _PAIR_SWAP += [2 * _i + 1, 2 * _i]


def _emit(nc, tc, mmdt, dram, tag=""):
    """Emit the whole per-core program. `dram` maps name -> DRAM AP."""
    xT = dram["xT"]
    wq, wk, wv, wo = dram["wq"], dram["wk"], dram["wv"], dram["wo"]
    cosE, sinE, maskneg = dram["cosE"], dram["sinE"], dram["maskneg"]
    yT = dram["yT"]

    need_round = mmdt != F32
    pvdt = BF16 if PV_BF16 else mmdt
    EXP = mybir.ActivationFunctionType.Exp

    import contextlib
    with contextlib.ExitStack() as ctx:
        # ---- persistent tiles -------------------------------------------
        per = ctx.enter_context(tc.tile_pool(name=f"per{tag}", bufs=1))
        KTt = [per.tile([128, S], mmdt, tag=f"KT{j}{tag}", name=f"KT{j}{tag}") for j in range(4)]
        vo = [per.tile([128, HPC, 65], pvdt, tag=f"vo{i}{tag}", name=f"vo{i}{tag}") for i in range(NKT)]
        ones_sb = per.tile([128, HPC], F32, tag=f"ones{tag}", name=f"ones{tag}")
        cos_sb = per.tile([128, S], F32, tag=f"cos{tag}", name=f"cos{tag}")
        sin_sb = per.tile([128, S], F32, tag=f"sin{tag}", name=f"sin{tag}")
        mask_f = per.tile([128, 2, 128], F32, tag=f"maskf{tag}", name=f"maskf{tag}")
        mask_sb = per.tile([128, 2, 128], pvdt, tag=f"mask{tag}", name=f"mask{tag}")
        wq_sb = per.tile([128, DT, JC], mmdt, tag=f"wq{tag}", name=f"wq{tag}")
        wo_sb = per.tile([128, 4, D], mmdt, tag=f"wo{tag}", name=f"wo{tag}")
        # per-chunk Q tiles, double-buffered across q-chunks
        pqt = ctx.enter_context(tc.tile_pool(name=f"pqt{tag}", bufs=2))
        pax = ctx.enter_context(tc.tile_pool(name=f"pax{tag}", bufs=XBUFS))
        pat = ctx.enter_context(tc.tile_pool(name=f"pat{tag}", bufs=RBUFS))

        xT_r = xT.rearrange("(dt p) s -> p dt s", p=128)

        def load_xc(sc):
            # x loads go on the Scalar engine's DMA queue so they never
            # head-of-line block the weight/output DMAs on the SP queue.
            ssl = slice(sc * SC, (sc + 1) * SC)
            xc = pax.tile([128, DT, SC], mmdt, tag=f"xc{tag}", name=f"xc{tag}")
            for dt in range(DT):
                nc.scalar.dma_start(out=xc[:, dt, :], in_=xT_r[:, dt, ssl])
            return xc

        def load_w(wt, w_ap):
            w_r = w_ap.rearrange("(dt p) j -> p dt j", p=128)
            for dt in range(DT):
                nc.sync.dma_start(out=wt[:, dt, :], in_=w_r[:, dt, :])

        def rope(ps, dst, ssl):
            # RoPE: dst = ps*cos + shuffle(ps)*sin.
            # GpSimd cannot touch PSUM, so DVE stages ps into SBUF; DVE
            # does shuffle + sin-mul, GpSimd cos-mul + the final add.
            qs = pat.tile([128, SC], F32, tag=f"ropes{tag}", name=f"ropes{tag}")
            qc_t = pat.tile([128, SC], F32, tag=f"ropec{tag}", name=f"ropec{tag}")
            if ROPE_SPLIT:
                pf = pat.tile([128, SC], F32, tag=f"ropef{tag}", name=f"ropef{tag}")
                nc.vector.tensor_copy(pf, ps)
                nc.vector.stream_shuffle(qs, pf, _PAIR_SWAP)
                nc.vector.tensor_mul(qs, qs, sin_sb[:, ssl])
                nc.gpsimd.tensor_mul(qc_t, pf, cos_sb[:, ssl])
                nc.gpsimd.tensor_add(dst, qc_t, qs)
            else:
                nc.vector.stream_shuffle(qs, ps, _PAIR_SWAP)
                nc.vector.tensor_mul(qs, qs, sin_sb[:, ssl])
                nc.vector.tensor_mul(qc_t, ps, cos_sb[:, ssl])
                nc.vector.tensor_add(dst, qc_t, qs)

        def proj_q(qc_idx, jt, xc, psum_pool, psum_tag, bufs=None):
            kw = {} if bufs is None else {"bufs": bufs}
            ps = psum_pool.tile([128, SC], F32, tag=psum_tag,
                                name=psum_tag, **kw)
            jl = slice(jt * 128, (jt + 1) * 128)
            for dt in range(DT):
                nc.tensor.matmul(ps, wq_sb[:, dt, jl], xc[:, dt, :],
                                 start=(dt == 0), stop=(dt == DT - 1))
            nqt = pqt.tile([128, SC], mmdt, tag=f"QTq{jt}{tag}",
                           name=f"QTq{jt}{tag}")
            rope(ps, nqt, slice(qc_idx * SC, (qc_idx + 1) * SC))
            return nqt

        # ---- phase A': K (all chunks), V (all chunks), Q chunk 0 --------
        with tc.tile_pool(name=f"pAw{tag}", bufs=1) as paw, \
             tc.tile_pool(name=f"pAps{tag}", bufs=8, space="PSUM") as paps:
            wk_sb = paw.tile([128, DT, JC], mmdt, tag=f"wk{tag}", name=f"wk{tag}")
            wv_sb = paw.tile([128, DT, JC], mmdt, tag=f"wv{tag}", name=f"wv{tag}")

            # K first: interleave wk and x0 per-dt so the dt-outer K matmuls
            # start after just one dt-slice of each has landed.
            wk_r = wk.rearrange("(dt p) j -> p dt j", p=128)
            xcK = pax.tile([128, DT, SC], mmdt, tag=f"xc{tag}", name=f"xc{tag}")
            for dt in range(DT):
                nc.sync.dma_start(out=wk_sb[:, dt, :], in_=wk_r[:, dt, :])
                nc.scalar.dma_start(out=xcK[:, dt, :], in_=xT_r[:, dt, 0:SC])
            nc.sync.dma_start(out=cos_sb, in_=cosE)
            nc.sync.dma_start(out=sin_sb, in_=sinE)
            load_w(wv_sb, wv)
            load_w(wq_sb, wq)
            wo_r = wo.rearrange("(hp p) m -> p hp m", p=128)
            for hp in range(4):
                nc.sync.dma_start(out=wo_sb[:, hp, :], in_=wo_r[:, hp, :])
            nc.sync.dma_start(out=mask_f.rearrange("p a b -> p (a b)"),
                              in_=maskneg)
            nc.vector.memset(ones_sb, 1.0)
            nc.scalar.copy(mask_sb, mask_f)
            # ones column of V is constant across the whole run: set it once.
            for i in range(NKT):
                o_dst = vo[i][:, :, 64:65]
                o_src = ones_sb.rearrange("p (h o) -> p h o", o=1)
                if need_round or pvdt != F32:
                    nc.scalar.copy(o_dst, o_src)
                else:
                    nc.vector.memset(o_dst, 1.0)

            # K projection + RoPE, dt-outer (4 PSUM tiles per chunk)
            for sc in range(NSC):
                ssl = slice(sc * SC, (sc + 1) * SC)
                xc = xcK if sc == 0 else load_xc(sc)
                qk = [paps.tile([128, SC], F32, tag=f"aps{tag}",
                                name=f"aps{tag}") for _ in range(4)]
                for dt in range(DT):
                    for jt in range(4):
                        jl = slice(jt * 128, (jt + 1) * 128)
                        nc.tensor.matmul(qk[jt], wk_sb[:, dt, jl],
                                         xc[:, dt, :],
                                         start=(dt == 0), stop=(dt == DT - 1))
                for jt in range(4):
                    rope(qk[jt], KTt[jt][:, ssl], ssl)

            # V projection, dt-outer
            for sc in range(NSC):
                xc = load_xc(sc)
                pv = [paps.tile([128, JC], F32, tag=f"aps{tag}",
                                name=f"aps{tag}") for _ in range(4)]
                for dt in range(DT):
                    for st in range(4):
                        sl = slice(st * 128, (st + 1) * 128)
                        nc.tensor.matmul(pv[st], xc[:, dt, sl],
                                         wv_sb[:, dt, :],
                                         start=(dt == 0), stop=(dt == DT - 1))
                for st in range(4):
                    vt = vo[sc * 4 + st]
                    pv_r = pv[st].rearrange("p (h j) -> p h j", h=HPC)
                    if VCOPY_ACT:
                        nc.scalar.copy(vt[:, :, 0:64], pv_r)
                    else:
                        nc.vector.tensor_copy(vt[:, :, 0:64], pv_r)

            # Q for chunk 0
            xc0 = load_xc(0)
            cur_QT = [proj_q(0, jt, xc0, paps, f"aps{tag}")
                      for jt in range(4)]

        # ---- phase B: attention + output projection, Q projected one
        # chunk ahead inside the hp loop ----------------------------------
        with tc.tile_pool(name=f"pBe{tag}", bufs=EBUFS) as pbe, \
             tc.tile_pool(name=f"pBt1{tag}", bufs=1) as pbt1, \
             tc.tile_pool(name=f"pBt{tag}", bufs=2) as pbt, \
             tc.tile_pool(name=f"pBo{tag}", bufs=OBUFS) as pbo, \
             tc.tile_pool(name=f"pBps{tag}", bufs=1, space="PSUM") as pbps, \
             tc.tile_pool(name=f"pBps2{tag}", bufs=2, space="PSUM") as pbps2:

            def emit_outproj(qsl_, oTs_, mts):
                for mt in mts:
                    yps = pbps2.tile([128, SC], F32, tag=f"yps{tag}",
                                     name=f"yps{tag}", bufs=YBUFS)
                    ml = slice(mt * 128, (mt + 1) * 128)
                    for hp_ in range(4):
                        nc.tensor.matmul(yps, wo_sb[:, hp_, ml], oTs_[hp_],
                                         start=(hp_ == 0), stop=(hp_ == 3))
                    ys = pbt.tile([128, SC], F32, tag=f"ys{tag}", name=f"ys{tag}")
                    nc.vector.tensor_copy(ys, yps)
                    nc.sync.dma_start(out=yT[ml, qsl_], in_=ys)

            prev = None  # (qsl, oTs) of the previous q-chunk, out-proj
            # deferred into the next chunk's hp loop so ready out-proj
            # matmuls fill PE while attention waits on exp/normalize.
            for qc in range(NSC):
                qsl = slice(qc * SC, (qc + 1) * SC)
                # prefetch x for the next chunk's Q projection
                xq = load_xc(qc + 1) if qc + 1 < NSC else None
                next_QT = [None] * 4
                oTs = []
                for hp in range(4):
                    QTh = cur_QT[hp]
                    pva = pbps.tile([65, SC], F32, tag=f"pva{tag}", name=f"pva{tag}")
                    pvb = pbps.tile([65, SC], F32, tag=f"pvb{tag}", name=f"pvb{tag}")
                    nkt = 4 * qc + 4
                    h0, h1 = 2 * hp, 2 * hp + 1
                    pending = None  # software pipeline: PV lags scores by 1
                    for kt in range(nkt):
                        ksl = slice(kt * KT, (kt + 1) * KT)
                        d = kt - 4 * qc
                        # diagonal tiles: only columns q >= 128*d are causally
                        # valid -- shrink scores/exp/PV to that range; the
                        # boundary 128-wide strip still needs the triangular
                        # mask.
                        cs = 128 * d if d > 0 else 0
                        vq = slice(cs, SC)
                        sc2 = pbps2.tile([128, 2, SC], F32, tag=f"sc2{tag}",
                                         name=f"sc2{tag}", bufs=SCBUFS)
                        sca, scb = sc2[:, 0, :], sc2[:, 1, :]
                        nc.tensor.matmul(sca[:, vq], KTt[hp][0:64, ksl],
                                         QTh[0:64, vq],
                                         start=True, stop=True,
                                         tile_position=(0, 0))
                        nc.tensor.matmul(scb[:, vq], KTt[hp][64:128, ksl],
                                         QTh[64:128, vq],
                                         start=True, stop=True,
                                         tile_position=(64, 0))
                        e2 = pbe.tile([128, 2, SC], pvdt, tag=f"e2{tag}",
                                      name=f"e2{tag}")
                        nc.scalar.activation(e2[:, :, vq], sc2[:, :, vq],
                                             EXP, scale=0.125)
                        if d >= 0:
                            # causal boundary strip: zero the exp of the
                            # invalid (q < k) positions with a 0/1 mask.
                            # GpSimd can do this since e2 lives in SBUF.
                            bs = slice(cs, cs + 128)
                            if MASK_GPS:
                                nc.gpsimd.tensor_mul(e2[:, :, bs],
                                                     e2[:, :, bs], mask_sb)
                            else:
                                nc.vector.tensor_mul(e2[:, :, bs],
                                                     e2[:, :, bs], mask_sb)
                        ea, eb = e2[:, 0, :], e2[:, 1, :]
                        if pending is not None:
                            pkt, pea, peb, pvq = pending
                            nc.tensor.matmul(pva[:, pvq], vo[pkt][:, h0, :],
                                             pea[:, pvq],
                                             start=(pkt == 0), stop=False)
                            nc.tensor.matmul(pvb[:, pvq], vo[pkt][:, h1, :],
                                             peb[:, pvq],
                                             start=(pkt == 0), stop=False)
                        pending = (kt, ea, eb, vq)
                    pkt, pea, peb, pvq = pending
                    nc.tensor.matmul(pva[:, pvq], vo[pkt][:, h0, :],
                                     pea[:, pvq],
                                     start=(pkt == 0), stop=True)
                    nc.tensor.matmul(pvb[:, pvq], vo[pkt][:, h1, :],
                                     peb[:, pvq],
                                     start=(pkt == 0), stop=True)
                    # normalize: oT[j, q] = pv[j, q] / denom[q].
                    # Stage PSUM -> SBUF (partition-aligned) so pva/pvb free
                    # early; reciprocal reads the PSUM denom row directly
                    # (in@p64 -> out@p0 is valid for single-input DVE ops);
                    # broadcast to 64 partitions at base 0; head B's rows are
                    # DMA-relocated to 64:128 (engine ops cannot cross-base).
                    o2 = pbt1.tile([65, 2, SC], F32, tag=f"o2{tag}", name=f"o2{tag}")
                    nc.vector.tensor_copy(o2[:, 0, :], pva)
                    nc.vector.tensor_copy(o2[:, 1, :], pvb)
                    d2 = pbt1.tile([1, 2, SC], F32, tag=f"d2{tag}", name=f"d2{tag}")
                    nc.vector.reciprocal(d2[:, 0, :], pva[64:65, :])
                    nc.vector.reciprocal(d2[:, 1, :], pvb[64:65, :])
                    bc = pbt1.tile([64, 2, SC], F32, tag=f"bc{tag}", name=f"bc{tag}")
                    nc.gpsimd.partition_broadcast(bc[:, 0, :], d2[:, 0, :])
                    nc.gpsimd.partition_broadcast(bc[:, 1, :], d2[:, 1, :])
                    oT = pbo.tile([128, SC], mmdt, tag=f"oT{hp}{tag}", name=f"oT{hp}{tag}")
                    tmpB = pbt.tile([64, SC], mmdt, tag=f"tmpB{tag}", name=f"tmpB{tag}")
                    nc.vector.tensor_mul(oT[0:64, :], o2[0:64, 0, :], bc[:, 0, :])
                    nc.vector.tensor_mul(tmpB, o2[0:64, 1, :], bc[:, 1, :])
                    nc.sync.dma_start(out=oT[64:128, :], in_=tmpB)
                    oTs.append(oT)
                    if prev is not None:
                        emit_outproj(prev[0], prev[1], [2 * hp, 2 * hp + 1])
                    if xq is not None:
                        # project next chunk's Q for this head pair; PSUM
                        # comes from the yps tag (same shape, spare slots)
                        next_QT[hp] = proj_q(qc + 1, hp, xq, pbps2,
                                             f"yps{tag}", YBUFS)
                prev = (qsl, oTs)
                if xq is not None:
                    cur_QT = next_QT
            emit_outproj(prev[0], prev[1], range(8))


_BUILT = {}


def build_nc(mmdt_name=MM_DTYPE, repeat=1):
    key = (mmdt_name, repeat)
    if key in _BUILT:
        return _BUILT[key]
    mmdt = {"f32": F32, "f32r": F32R}[mmdt_name]
    nc = bacc.Bacc("TRN2", target_bir_lowering=False, debug=False,
                   num_devices=N_CORES)
    dram = {
        "xT": nc.dram_tensor("xT", [D, S], mmdt, kind="ExternalInput").ap(),
        "wq": nc.dram_tensor("wq", [D, JC], mmdt, kind="ExternalInput").ap(),
        "wk": nc.dram_tensor("wk", [D, JC], mmdt, kind="ExternalInput").ap(),
        "wv": nc.dram_tensor("wv", [D, JC], mmdt, kind="ExternalInput").ap(),
        "wo": nc.dram_tensor("wo", [JC, D], mmdt, kind="ExternalInput").ap(),
        "cosE": nc.dram_tensor("cosE", [128, S], F32,
                               kind="ExternalInput").ap(),
        "sinE": nc.dram_tensor("sinE", [128, S], F32,
                               kind="ExternalInput").ap(),
        "maskneg": nc.dram_tensor("maskneg", [128, 256], F32,
                                  kind="ExternalInput").ap(),
        "yT": nc.dram_tensor("yT", [D, S], F32, kind="ExternalOutput").ap(),
    }
    with tile.TileContext(nc) as tc:
        for r in range(repeat):
            _emit(nc, tc, mmdt, dram, tag=f"r{r}" if repeat > 1 else "")
    nc.compile()
    _BUILT[key] = nc
    return nc


def _round_f32r(a):
    """Round-to-nearest onto the f32r grid (fp32 with low 12 mantissa bits 0)."""
    b = np.ascontiguousarray(a, np.float32).view(np.uint32).astype(np.uint64)
    b = (b + 0x800 + ((b >> 12) & 1)) & 0xFFFFF000
    return b.astype(np.uint32).view(np.float32)


def _host_prep(x, pos_ids, Wq, Wk, Wv, Wo, cos, sin, mmdt_name=None):
    """Build the 8 per-core input maps."""
    if mmdt_name is None:
        mmdt_name = MM_DTYPE
    rnd = _round_f32r if mmdt_name == "f32r" else (lambda a: a)
    x = np.asarray(x, dtype=np.float32)
    pos_ids = np.asarray(pos_ids)
    cos = np.asarray(cos, dtype=np.float32)
    sin = np.asarray(sin, dtype=np.float32)
    freq_idx = np.tile(np.repeat(np.arange(DK // 2), 2), 2)  # [128]
    sign = np.where((np.arange(128) % 2) == 0, -1.0, 1.0).astype(np.float32)

    # universal triangular boundary mask: 1 if q >= p else 0 (multiplied
    # into exp(scores) post-activation); two side-by-side copies (one per
    # head in the merged [128, 2, 128] tile)
    p = np.arange(128)[:, None]
    q = np.arange(128)[None, :]
    mask1 = np.where(q >= p, 1.0, 0.0).astype(np.float32)
    mask = np.concatenate([mask1, mask1], axis=1)  # [128, 256]

    in_maps = []
    for c in range(N_CORES):
        b, g = c // 2, c % 2
        hs = slice(64 * HPC * g, 64 * HPC * g + JC)
        pos = pos_ids[b].astype(np.int64)
        cosT = cos[pos].T  # [32, S]
        sinT = sin[pos].T
        cosE = np.ascontiguousarray(cosT[freq_idx])           # [128, S]
        sinE = np.ascontiguousarray(sinT[freq_idx] * sign[:, None])
        in_maps.append({
            "xT": rnd(np.ascontiguousarray(x[b].T)),
            "wq": rnd(np.ascontiguousarray(Wq[hs, :].T)),
            "wk": rnd(np.ascontiguousarray(Wk[hs, :].T)),
            "wv": rnd(np.ascontiguousarray(Wv[hs, :].T)),
            "wo": rnd(np.ascontiguousarray(Wo[:, hs].T)),
            "cosE": cosE,
            "sinE": sinE,
            "maskneg": mask,
        })
    return in_maps


def kernel(x, pos_ids, Wq, Wk, Wv, Wo, cos, sin):
    nc = build_nc()
    in_maps = _host_prep(x, pos_ids, Wq, Wk, Wv, Wo, cos, sin)
    res = run_bass_kernel_spmd(nc, in_maps, list(range(N_CORES)))
    out = np.empty((B, S, D), dtype=np.float32)
    for b in range(B):
        yT = res.results[2 * b]["yT"] + res.results[2 * b + 1]["yT"]
        out[b] = yT.T
    return out


if __name__ == "__main__":
    t0 = time.time()
    nc = build_nc()
    print(f"build+compile: {time.time()-t0:.1f}s", flush=True)


# revision 24
# speedup vs baseline: 1.0384x; 1.0384x over previous
"""Causal multi-head self-attention with RoPE on 8 TRN2 NeuronCores.

Sharding: batch(4) x head-group(2) -> 8 cores. Core c handles batch c//2 and
heads [8*(c%2), 8*(c%2)+8). Each core computes its partial output projection
(sum over its 8 heads' contribution); the host adds the two head-group
partials per batch. No device collectives needed.

Fully streamed single-pass schedule: after a small init (Q/K/V of chunk 0),
chunk qc's attention runs while chunk qc+1's Q/K/V projections are emitted
between its head-pair blocks -- causality guarantees attention(qc) only
needs chunks <= qc. The output projection of chunk qc-1 is likewise deferred
into chunk qc's loop so ready matmuls keep PE fed while exp/normalize drain.

Dtypes: matmul operands are bf16 (x resident in SBUF, loaded once; weights,
Q/K/V, P=exp(scores), output proj) -- on the PE this costs the same
cycles/row as f32r but halves SBUF/DMA and avoids the f32r narrow-tile
penalty. All accumulation is f32 in PSUM; softmax normalize runs in f32;
the final y is f32. Measured rel err vs the f32 reference ~= 4e-3.

On-chip layout: sequence on the free dimension everywhere.
  - Q^T/K^T [j, s] from projection matmuls (lhsT = W^T slices, rhs = x^T),
    RoPE via partition pair-swap (stream_shuffle) + cos/sin tables; the
    elementwise work is split DVE (stage/shuffle/sin-mul) / GpSimd
    (cos-mul/add).
  - scores S^T [k, q] per k-tile into a merged [128, 2, SC] PSUM tile (both
    heads of the pair), one exp per k-tile on ScalarE (|scores|/8 <= ~15 so
    no max subtraction), causal boundary strips zeroed post-exp with a 0/1
    mask on GpSimd.
  - P@V contracts k on partitions; a ones-row appended to V yields the
    softmax denominator from the same matmul (M=65).
  - output projection contracts the 512 head-dims -> partial y^T [1024, s].
"""

import os
import sys
import time

for _p in ("/opt/trn_rl_repo", "/root/.axon_site/_ro/trn_rl_repo"):
    if _p not in sys.path and os.path.isdir(_p):
        sys.path.insert(0, _p)

import numpy as np
import concourse.bass as bass
import concourse.bacc as bacc
import concourse.mybir as mybir
import concourse.tile as tile
from concourse.bass_utils import run_bass_kernel_spmd

F32 = mybir.dt.float32
F32R = mybir.dt.float32r
BF16 = mybir.dt.bfloat16

B, S, D = 4, 2048, 1024
H, DK = 16, 64
HPC = 8            # heads per core
JC = HPC * DK      # 512 head-dims per core
N_CORES = 8
SC = 512           # q-chunk width (moving free dim)
NSC = S // SC      # 4
KT = 128           # k-tile (scores partition dim)
NKT = S // KT      # 16
DT = D // 128      # 8 contraction tiles for projections

# matmul operand dtype: "bf16" (fast, rel err ~4e-3), "f32r", or "f32"
MM_DTYPE = os.environ.get("KERNEL_MM_DTYPE", "bf16")
EBUFS = int(os.environ.get("KV_EBUFS", "8"))
SCBUFS = int(os.environ.get("KV_SCBUFS", "2"))
YBUFS = int(os.environ.get("KV_YBUFS", "2"))
XBUFS = int(os.environ.get("KV_XBUFS", "2"))
RBUFS = int(os.environ.get("KV_RBUFS", "2"))
OBUFS = int(os.environ.get("KV_OBUFS", "2"))
ROPE_SPLIT = os.environ.get("KV_ROPE_SPLIT", "1") == "1"
VCOPY_ACT = os.environ.get("KV_VCOPY_ACT", "1") == "1"
MASK_GPS = os.environ.get("KV_MASK_GPS", "1") == "1"


_PAIR_SWAP = []
for _i in range(16):
    _PAIR_SWAP += [2 * _i + 1, 2 * _i]


def _emit(nc, tc, mmdt, dram, tag=""):
    """Emit the whole per-core program. `dram` maps name -> DRAM AP."""
    xT = dram["xT"]
    wq, wk, wv, wo = dram["wq"], dram["wk"], dram["wv"], dram["wo"]
    cosE, sinE, maskneg = dram["cosE"], dram["sinE"], dram["maskneg"]
    yT = dram["yT"]

    pvdt = mmdt
    EXP = mybir.ActivationFunctionType.Exp

    import contextlib
    with contextlib.ExitStack() as ctx:
        # ---- persistent tiles -------------------------------------------
        per = ctx.enter_context(tc.tile_pool(name=f"per{tag}", bufs=1))
        KTt = [per.tile([128, S], mmdt, tag=f"KT{j}{tag}", name=f"KT{j}{tag}") for j in range(4)]
        vo = [per.tile([128, HPC, 65], pvdt, tag=f"vo{i}{tag}", name=f"vo{i}{tag}") for i in range(NKT)]
        ones_sb = per.tile([128, HPC], F32, tag=f"ones{tag}", name=f"ones{tag}")
        cos_sb = per.tile([128, S], F32, tag=f"cos{tag}", name=f"cos{tag}")
        sin_sb = per.tile([128, S], F32, tag=f"sin{tag}", name=f"sin{tag}")
        mask_f = per.tile([128, 2, 128], F32, tag=f"maskf{tag}", name=f"maskf{tag}")
        mask_sb = per.tile([128, 2, 128], pvdt, tag=f"mask{tag}", name=f"mask{tag}")
        wq_sb = per.tile([128, DT, JC], mmdt, tag=f"wq{tag}", name=f"wq{tag}")
        wk_sb = per.tile([128, DT, JC], mmdt, tag=f"wk{tag}", name=f"wk{tag}")
        wv_sb = per.tile([128, DT, JC], mmdt, tag=f"wv{tag}", name=f"wv{tag}")
        wo_sb = per.tile([128, 4, D], mmdt, tag=f"wo{tag}", name=f"wo{tag}")
        # x^T resident in SBUF, loaded once (bf16 makes this affordable)
        xr = [per.tile([128, DT, SC], mmdt, tag=f"xr{i}{tag}", name=f"xr{i}{tag}")
              for i in range(NSC)]
        # per-chunk Q tiles, double-buffered across q-chunks
        pqt = ctx.enter_context(tc.tile_pool(name=f"pqt{tag}", bufs=2))
        pat = ctx.enter_context(tc.tile_pool(name=f"pat{tag}", bufs=RBUFS))
        pbe = ctx.enter_context(tc.tile_pool(name=f"pbe{tag}", bufs=EBUFS))
        pbt1 = ctx.enter_context(tc.tile_pool(name=f"pbt1{tag}", bufs=1))
        pbt = ctx.enter_context(tc.tile_pool(name=f"pbt{tag}", bufs=2))
        pbo = ctx.enter_context(tc.tile_pool(name=f"pbo{tag}", bufs=OBUFS))
        pbps = ctx.enter_context(tc.tile_pool(name=f"pbps{tag}", bufs=1, space="PSUM"))
        pbps2 = ctx.enter_context(tc.tile_pool(name=f"pbps2{tag}", bufs=2, space="PSUM"))

        xT_r = xT.rearrange("(dt p) s -> p dt s", p=128)

        def load_w(wt, w_ap):
            w_r = w_ap.rearrange("(dt p) j -> p dt j", p=128)
            for dt in range(DT):
                nc.sync.dma_start(out=wt[:, dt, :], in_=w_r[:, dt, :])

        def rope(ps, dst, ssl):
            # RoPE: dst = ps*cos + shuffle(ps)*sin.
            # GpSimd cannot touch PSUM, so DVE stages ps into SBUF; DVE
            # does shuffle + sin-mul, GpSimd cos-mul + the final add.
            qs = pat.tile([128, SC], F32, tag=f"ropes{tag}", name=f"ropes{tag}")
            qc_t = pat.tile([128, SC], F32, tag=f"ropec{tag}", name=f"ropec{tag}")
            if ROPE_SPLIT:
                pf = pat.tile([128, SC], F32, tag=f"ropef{tag}", name=f"ropef{tag}")
                nc.vector.tensor_copy(pf, ps)
                nc.vector.stream_shuffle(qs, pf, _PAIR_SWAP)
                nc.vector.tensor_mul(qs, qs, sin_sb[:, ssl])
                nc.gpsimd.tensor_mul(qc_t, pf, cos_sb[:, ssl])
                nc.gpsimd.tensor_add(dst, qc_t, qs)
            else:
                nc.vector.stream_shuffle(qs, ps, _PAIR_SWAP)
                nc.vector.tensor_mul(qs, qs, sin_sb[:, ssl])
                nc.vector.tensor_mul(qc_t, ps, cos_sb[:, ssl])
                nc.vector.tensor_add(dst, qc_t, qs)

        def misc_psum():
            # shares the yps tag: out-proj accumulation and Q/K/V projection
            # accumulation are both sporadic [128, SC] f32 users
            return pbps2.tile([128, SC], F32, tag=f"yps{tag}",
                              name=f"yps{tag}", bufs=YBUFS)

        def proj_q(qc_idx, jt):
            ps = misc_psum()
            jl = slice(jt * 128, (jt + 1) * 128)
            for dt in range(DT):
                nc.tensor.matmul(ps, wq_sb[:, dt, jl], xr[qc_idx][:, dt, :],
                                 start=(dt == 0), stop=(dt == DT - 1))
            nqt = pqt.tile([128, SC], mmdt, tag=f"QTq{jt}{tag}",
                           name=f"QTq{jt}{tag}")
            rope(ps, nqt, slice(qc_idx * SC, (qc_idx + 1) * SC))
            return nqt

        def proj_k(sc, jt):
            ps = misc_psum()
            jl = slice(jt * 128, (jt + 1) * 128)
            for dt in range(DT):
                nc.tensor.matmul(ps, wk_sb[:, dt, jl], xr[sc][:, dt, :],
                                 start=(dt == 0), stop=(dt == DT - 1))
            ssl = slice(sc * SC, (sc + 1) * SC)
            rope(ps, KTt[jt][:, ssl], ssl)

        def proj_v(sc, st):
            ps = misc_psum()
            sl = slice(st * 128, (st + 1) * 128)
            for dt in range(DT):
                nc.tensor.matmul(ps, xr[sc][:, dt, sl], wv_sb[:, dt, :],
                                 start=(dt == 0), stop=(dt == DT - 1))
            vt = vo[sc * 4 + st]
            ps_r = ps.rearrange("p (h j) -> p h j", h=HPC)
            if VCOPY_ACT:
                nc.scalar.copy(vt[:, :, 0:64], ps_r)
            else:
                nc.vector.tensor_copy(vt[:, :, 0:64], ps_r)

        def emit_outproj(qsl_, oTs_, mts):
            for mt in mts:
                yps = pbps2.tile([128, SC], F32, tag=f"yps{tag}",
                                 name=f"yps{tag}", bufs=YBUFS)
                ml = slice(mt * 128, (mt + 1) * 128)
                for hp_ in range(4):
                    nc.tensor.matmul(yps, wo_sb[:, hp_, ml], oTs_[hp_],
                                     start=(hp_ == 0), stop=(hp_ == 3))
                ys = pbt.tile([128, SC], F32, tag=f"ys{tag}", name=f"ys{tag}")
                nc.vector.tensor_copy(ys, yps)
                nc.sync.dma_start(out=yT[ml, qsl_], in_=ys)

        # ---- DMA issue: weights on the SP queue, x on the Scalar queue
        # (separate queues avoid head-of-line blocking; transfers share the
        # DMA engine pool). wq/x0 first: the init Q0 projection needs them.
        load_w(wq_sb, wq)
        for dt in range(DT):
            nc.scalar.dma_start(out=xr[0][:, dt, :], in_=xT_r[:, dt, 0:SC])
        load_w(wk_sb, wk)
        nc.sync.dma_start(out=cos_sb, in_=cosE)
        nc.sync.dma_start(out=sin_sb, in_=sinE)
        for i in range(1, NSC):
            ssl = slice(i * SC, (i + 1) * SC)
            for dt in range(DT):
                nc.scalar.dma_start(out=xr[i][:, dt, :], in_=xT_r[:, dt, ssl])
        load_w(wv_sb, wv)
        wo_r = wo.rearrange("(hp p) m -> p hp m", p=128)
        for hp in range(4):
            nc.sync.dma_start(out=wo_sb[:, hp, :], in_=wo_r[:, hp, :])
        nc.sync.dma_start(out=mask_f.rearrange("p a b -> p (a b)"),
                          in_=maskneg)
        nc.vector.memset(ones_sb, 1.0)
        nc.scalar.copy(mask_sb, mask_f)
        # ones column of V is constant across the whole run: set it once.
        for i in range(NKT):
            nc.scalar.copy(vo[i][:, :, 64:65],
                           ones_sb.rearrange("p (h o) -> p h o", o=1))

        # ---- init: chunk 0's Q, K, V ------------------------------------
        cur_QT = [proj_q(0, jt) for jt in range(4)]
        for jt in range(4):
            proj_k(0, jt)
        for st in range(4):
            proj_v(0, st)

        # ---- main loop: attention(qc) + projections(qc+1) interleaved ---
        prev = None  # (qsl, oTs) of the previous q-chunk; its out-proj is
        # deferred into this chunk's hp loop so ready matmuls fill PE.
        for qc in range(NSC):
            qsl = slice(qc * SC, (qc + 1) * SC)
            next_QT = [None] * 4
            oTs = []
            for hp in range(4):
                QTh = cur_QT[hp]
                pva = pbps.tile([65, SC], F32, tag=f"pva{tag}", name=f"pva{tag}")
                pvb = pbps.tile([65, SC], F32, tag=f"pvb{tag}", name=f"pvb{tag}")
                nkt = 4 * qc + 4
                h0, h1 = 2 * hp, 2 * hp + 1
                pending = None  # software pipeline: PV lags scores by 1
                for kt in range(nkt):
                    ksl = slice(kt * KT, (kt + 1) * KT)
                    d = kt - 4 * qc
                    # diagonal tiles: only columns q >= 128*d are causally
                    # valid -- shrink scores/exp/PV to that range; the
                    # boundary 128-wide strip still needs the triangular
                    # mask.
                    cs = 128 * d if d > 0 else 0
                    vq = slice(cs, SC)
                    sc2 = pbps2.tile([128, 2, SC], F32, tag=f"sc2{tag}",
                                     name=f"sc2{tag}", bufs=SCBUFS)
                    sca, scb = sc2[:, 0, :], sc2[:, 1, :]
                    nc.tensor.matmul(sca[:, vq], KTt[hp][0:64, ksl],
                                     QTh[0:64, vq],
                                     start=True, stop=True,
                                     tile_position=(0, 0))
                    nc.tensor.matmul(scb[:, vq], KTt[hp][64:128, ksl],
                                     QTh[64:128, vq],
                                     start=True, stop=True,
                                     tile_position=(64, 0))
                    e2 = pbe.tile([128, 2, SC], pvdt, tag=f"e2{tag}",
                                  name=f"e2{tag}")
                    nc.scalar.activation(e2[:, :, vq], sc2[:, :, vq],
                                         EXP, scale=0.125)
                    if d >= 0:
                        # causal boundary strip: zero the exp of the
                        # invalid (q < k) positions with a 0/1 mask.
                        # GpSimd can do this since e2 lives in SBUF.
                        bs = slice(cs, cs + 128)
                        if MASK_GPS:
                            nc.gpsimd.tensor_mul(e2[:, :, bs],
                                                 e2[:, :, bs], mask_sb)
                        else:
                            nc.vector.tensor_mul(e2[:, :, bs],
                                                 e2[:, :, bs], mask_sb)
                    ea, eb = e2[:, 0, :], e2[:, 1, :]
                    if pending is not None:
                        pkt, pea, peb, pvq = pending
                        nc.tensor.matmul(pva[:, pvq], vo[pkt][:, h0, :],
                                         pea[:, pvq],
                                         start=(pkt == 0), stop=False)
                        nc.tensor.matmul(pvb[:, pvq], vo[pkt][:, h1, :],
                                         peb[:, pvq],
                                         start=(pkt == 0), stop=False)
                    pending = (kt, ea, eb, vq)
                pkt, pea, peb, pvq = pending
                nc.tensor.matmul(pva[:, pvq], vo[pkt][:, h0, :],
                                 pea[:, pvq],
                                 start=(pkt == 0), stop=True)
                nc.tensor.matmul(pvb[:, pvq], vo[pkt][:, h1, :],
                                 peb[:, pvq],
                                 start=(pkt == 0), stop=True)
                # normalize: oT[j, q] = pv[j, q] / denom[q].
                # Stage PSUM -> SBUF (partition-aligned) so pva/pvb free
                # early; reciprocal reads the PSUM denom row directly
                # (in@p64 -> out@p0 is valid for single-input DVE ops);
                # broadcast to 64 partitions at base 0; head B's rows are
                # DMA-relocated to 64:128 (engine ops cannot cross-base).
                o2 = pbt1.tile([65, 2, SC], F32, tag=f"o2{tag}", name=f"o2{tag}")
                nc.vector.tensor_copy(o2[:, 0, :], pva)
                nc.vector.tensor_copy(o2[:, 1, :], pvb)
                d2 = pbt1.tile([1, 2, SC], F32, tag=f"d2{tag}", name=f"d2{tag}")
                nc.vector.reciprocal(d2[:, 0, :], pva[64:65, :])
                nc.vector.reciprocal(d2[:, 1, :], pvb[64:65, :])
                bc = pbt1.tile([64, 2, SC], F32, tag=f"bc{tag}", name=f"bc{tag}")
                nc.gpsimd.partition_broadcast(bc[:, 0, :], d2[:, 0, :])
                nc.gpsimd.partition_broadcast(bc[:, 1, :], d2[:, 1, :])
                oT = pbo.tile([128, SC], mmdt, tag=f"oT{hp}{tag}", name=f"oT{hp}{tag}")
                tmpB = pbt.tile([64, SC], mmdt, tag=f"tmpB{tag}", name=f"tmpB{tag}")
                nc.vector.tensor_mul(oT[0:64, :], o2[0:64, 0, :], bc[:, 0, :])
                nc.vector.tensor_mul(tmpB, o2[0:64, 1, :], bc[:, 1, :])
                nc.sync.dma_start(out=oT[64:128, :], in_=tmpB)
                oTs.append(oT)
                if prev is not None:
                    emit_outproj(prev[0], prev[1], [2 * hp, 2 * hp + 1])
                if qc + 1 < NSC:
                    # stream chunk qc+1's projections between hp blocks
                    next_QT[hp] = proj_q(qc + 1, hp)
                    proj_k(qc + 1, hp)
                    proj_v(qc + 1, hp)
            prev = (qsl, oTs)
            if qc + 1 < NSC:
                cur_QT = next_QT
        emit_outproj(prev[0], prev[1], range(8))


_BUILT = {}


def build_nc(mmdt_name=MM_DTYPE, repeat=1):
    key = (mmdt_name, repeat)
    if key in _BUILT:
        return _BUILT[key]
    mmdt = {"f32": F32, "f32r": F32R, "bf16": BF16}[mmdt_name]
    nc = bacc.Bacc("TRN2", target_bir_lowering=False, debug=False,
                   num_devices=N_CORES)
    dram = {
        "xT": nc.dram_tensor("xT", [D, S], mmdt, kind="ExternalInput").ap(),
        "wq": nc.dram_tensor("wq", [D, JC], mmdt, kind="ExternalInput").ap(),
        "wk": nc.dram_tensor("wk", [D, JC], mmdt, kind="ExternalInput").ap(),
        "wv": nc.dram_tensor("wv", [D, JC], mmdt, kind="ExternalInput").ap(),
        "wo": nc.dram_tensor("wo", [JC, D], mmdt, kind="ExternalInput").ap(),
        "cosE": nc.dram_tensor("cosE", [128, S], F32,
                               kind="ExternalInput").ap(),
        "sinE": nc.dram_tensor("sinE", [128, S], F32,
                               kind="ExternalInput").ap(),
        "maskneg": nc.dram_tensor("maskneg", [128, 256], F32,
                                  kind="ExternalInput").ap(),
        "yT": nc.dram_tensor("yT", [D, S], F32, kind="ExternalOutput").ap(),
    }
    with tile.TileContext(nc) as tc:
        for r in range(repeat):
            _emit(nc, tc, mmdt, dram, tag=f"r{r}" if repeat > 1 else "")
    nc.compile()
    _BUILT[key] = nc
    return nc


def _round_f32r(a):
    """Round-to-nearest onto the f32r grid (fp32 with low 12 mantissa bits 0)."""
    b = np.ascontiguousarray(a, np.float32).view(np.uint32).astype(np.uint64)
    b = (b + 0x800 + ((b >> 12) & 1)) & 0xFFFFF000
    return b.astype(np.uint32).view(np.float32)


def _to_bf16(a):
    import ml_dtypes
    return np.ascontiguousarray(a, np.float32).astype(ml_dtypes.bfloat16)


def _host_prep(x, pos_ids, Wq, Wk, Wv, Wo, cos, sin, mmdt_name=None):
    """Build the 8 per-core input maps."""
    if mmdt_name is None:
        mmdt_name = MM_DTYPE
    if mmdt_name == "f32r":
        rnd = _round_f32r
    elif mmdt_name == "bf16":
        rnd = _to_bf16
    else:
        rnd = lambda a: a
    x = np.asarray(x, dtype=np.float32)
    pos_ids = np.asarray(pos_ids)
    cos = np.asarray(cos, dtype=np.float32)
    sin = np.asarray(sin, dtype=np.float32)
    freq_idx = np.tile(np.repeat(np.arange(DK // 2), 2), 2)  # [128]
    sign = np.where((np.arange(128) % 2) == 0, -1.0, 1.0).astype(np.float32)

    # universal triangular boundary mask: 1 if q >= p else 0 (multiplied
    # into exp(scores) post-activation); two side-by-side copies (one per
    # head in the merged [128, 2, 128] tile)
    p = np.arange(128)[:, None]
    q = np.arange(128)[None, :]
    mask1 = np.where(q >= p, 1.0, 0.0).astype(np.float32)
    mask = np.concatenate([mask1, mask1], axis=1)  # [128, 256]

    in_maps = []
    for c in range(N_CORES):
        b, g = c // 2, c % 2
        hs = slice(64 * HPC * g, 64 * HPC * g + JC)
        pos = pos_ids[b].astype(np.int64)
        cosT = cos[pos].T  # [32, S]
        sinT = sin[pos].T
        cosE = np.ascontiguousarray(cosT[freq_idx])           # [128, S]
        sinE = np.ascontiguousarray(sinT[freq_idx] * sign[:, None])
        in_maps.append({
            "xT": rnd(np.ascontiguousarray(x[b].T)),
            "wq": rnd(np.ascontiguousarray(Wq[hs, :].T)),
            "wk": rnd(np.ascontiguousarray(Wk[hs, :].T)),
            "wv": rnd(np.ascontiguousarray(Wv[hs, :].T)),
            "wo": rnd(np.ascontiguousarray(Wo[:, hs].T)),
            "cosE": cosE,
            "sinE": sinE,
            "maskneg": mask,
        })
    return in_maps


def kernel(x, pos_ids, Wq, Wk, Wv, Wo, cos, sin):
    nc = build_nc()
    in_maps = _host_prep(x, pos_ids, Wq, Wk, Wv, Wo, cos, sin)
    res = run_bass_kernel_spmd(nc, in_maps, list(range(N_CORES)))
    out = np.empty((B, S, D), dtype=np.float32)
    for b in range(B):
        yT = res.results[2 * b]["yT"] + res.results[2 * b + 1]["yT"]
        out[b] = yT.T
    return out


if __name__ == "__main__":
    t0 = time.time()
    nc = build_nc()
    print(f"build+compile: {time.time()-t0:.1f}s", flush=True)
